# revision 6
# baseline (speedup 1.0000x reference)
"""GraphSAGE-style mean-aggregator encoder on Trainium2, 8-core SPMD.

Computation (per the reference):
    neigh = features[neigh_idx].mean(1)         # [B, F]
    self_ = features[nodes]                     # [B, F]
    out   = relu(W @ concat(self_, neigh).T)    # [E, B]

Sharding: data-parallel over the node batch B=16384 -> 2048 nodes/core;
features + (pre-transposed, pre-scaled) weight replicated per core.

The gather is the whole problem: 22528 random 2KB-class rows per core.
HW facts (probed on this silicon, stock SWDGE ucode):
  - indirect DMA (INDIRECT1D) reads exactly ONE int32 index per dst
    partition per instruction; multi-index offset APs are ignored
    ([P,W] pulls W *consecutive* rows) or fault ([16,N/16]).  The
    22528-row gather therefore needs >= 176 instructions.
  - each instruction costs ~1089ns SWDGE descriptor-gen on the Pool Q7
    + ~309ns sequencer dispatch -> a hard ~1.40us/instr cadence,
    independent of payload dtype (fp32 and bf16 measured identical).
    176 x 1.40us = 246us is the per-core floor; everything else must
    hide under it.
  - InstDMAGatherAnt (batched int16 gather w/ queue_num) faults this
    runtime (mlp ucode library unavailable) - not usable.
  - CCE compute_op=add in-DMA accumulation works (incl bf16->f32 cast)
    but costs ~1.75us gen/instr and serializes gen-after-drain: slower
    than DVE adds.

So this kernel runs the 176-gather stream at its natural cadence and
keeps every other engine far under it, in bf16:
  - features/W/identity bf16 (gather drain halves; PE 1-pass; DVE 2x)
  - per 128-node tile: 11 gathers; self-chunk PE transposes fire on the
    first gather, DVE chain-adds the 10 neighbors (1/S folded into W's
    neighbor half), nsum transposes, 8 accumulating bf16 matmuls into
    fp32 PSUM, ACT relu, per-tile store (all hidden under the stream).

Measured on 8xTRN2 (NTFF profile): 270.0us, rel err 2.7e-03
(fp32 everywhere was the 282.9us baseline; tolerance is 2e-2).
"""

import numpy as np
import ml_dtypes
from contextlib import ExitStack

import concourse.bass as bass
import concourse.mybir as mybir
import concourse.tile as tile
from concourse import bacc
from concourse.bass_utils import run_bass_kernel_spmd

NCORES = 8
B = 16384
BC = B // NCORES  # 2048 nodes per core
S = 10            # neighbor samples
J = S + 1         # gathered rows per node (self + neighbors)
F = 512           # feature dim
E = 128           # embed dim
NNODES = 200000
P = 128
TILES = BC // P   # 16
IDXW = 16         # padded width of the packed index rows
BF = ml_dtypes.bfloat16

_CACHE = {}


def build_nc():
    nc = bacc.Bacc(
        "TRN2",
        target_bir_lowering=False,
        debug=False,
        num_devices=NCORES,
    )

    gidx = nc.dram_tensor("gidx", [P, TILES * IDXW], mybir.dt.int32, kind="ExternalInput").ap()
    features = nc.dram_tensor(
        "features", [NNODES, F], mybir.dt.bfloat16, kind="ExternalInput"
    ).ap()
    # host-preprocessed: W^T with the neighbor half pre-scaled by 1/S -> [2F, E]
    wt = nc.dram_tensor("wt", [2 * F, E], mybir.dt.bfloat16, kind="ExternalInput").ap()
    ident = nc.dram_tensor("ident", [P, P], mybir.dt.bfloat16, kind="ExternalInput").ap()
    out = nc.dram_tensor("out", [E, BC], mybir.dt.float32, kind="ExternalOutput").ap()

    KCHUNKS = 2 * F // P  # 8

    with tile.TileContext(nc) as tc, ExitStack() as ctx:
        consts = ctx.enter_context(tc.tile_pool(name="consts", bufs=1))
        stpool = ctx.enter_context(tc.tile_pool(name="stpool", bufs=1))
        gpool = ctx.enter_context(tc.tile_pool(name="gpool", bufs=4))
        spool = ctx.enter_context(tc.tile_pool(name="spool", bufs=3))
        ctpool = ctx.enter_context(tc.tile_pool(name="ctpool", bufs=12))
        psum_t = ctx.enter_context(tc.tile_pool(name="psum_t", bufs=4, space="PSUM"))
        psum_o = ctx.enter_context(tc.tile_pool(name="psum_o", bufs=2, space="PSUM"))

        # indices first: the staging copies (and thus the gather pipeline)
        # depend on them. gidx is host-prelaid as [P, TILES*IDXW] so this is
        # one linear DMA.
        idx_all = consts.tile([P, TILES * IDXW], mybir.dt.int32)
        nc.sync.dma_start(out=idx_all[:], in_=gidx[:])

        identity = consts.tile([P, P], mybir.dt.bfloat16)
        nc.sync.dma_start(out=identity[:], in_=ident[:])

        # W^T chunks: wt dram rows (k p) -> sbuf [p, (k e)]
        wt_sb = consts.tile([P, KCHUNKS * E], mybir.dt.bfloat16)
        nc.sync.dma_start(
            out=wt_sb[:].rearrange("p (k e) -> p k e", k=KCHUNKS),
            in_=wt.rearrange("(k p) e -> p k e", k=KCHUNKS),
        )

        out_sb = consts.tile([E, BC], mybir.dt.float32)

        # Index staging is just-in-time: a big upfront DVE copy burst
        # contends with the first gathers' SBUF port, so only tiles 0-2
        # stage in the prologue; tile t+3 stages inside tile t's body.
        iview = idx_all[:].rearrange("p (t w) -> p t w", t=TILES)
        stages = [
            [
                stpool.tile([P, 1], mybir.dt.int32, tag=f"st{t}_{j}",
                            name=f"st{t}_{j}")
                for j in range(J)
            ]
            for t in range(TILES)
        ]

        def stage_tile(t):
            for j in range(J):
                nc.vector.tensor_copy(
                    out=stages[t][j][:], in_=iview[:, t, j : j + 1])

        for t in range(3):
            stage_tile(t)

        # Per-tile pipeline: gathers -> (self transposes | DVE adds ->
        # nsum transposes) -> 8 accumulating matmuls -> relu -> store.
        # Per-tile matmuls cost extra LoadStationary on the idle PE but keep
        # the post-last-gather critical path to one tile's worth of work.
        for t in range(TILES):
            # one single-index gather per sample into its own whole tile
            gs = []
            for j in range(J):
                gj = gpool.tile(
                    [P, F], mybir.dt.bfloat16, tag=f"g{j}", bufs=4,
                    name=f"g{t}_{j}",
                )
                off = idx_all[:, 0:1] if (t == 0 and j == 0) else stages[t][j][:]
                nc.gpsimd.indirect_dma_start(
                    out=gj[:],
                    out_offset=None,
                    in_=features[:],
                    in_offset=bass.IndirectOffsetOnAxis(ap=off, axis=0),
                )
                gs.append(gj)

            cts = [
                ctpool.tile([P, P], mybir.dt.bfloat16, tag=f"ct{k}", bufs=2,
                            name=f"ct{t}_{k}")
                for k in range(KCHUNKS)
            ]
            # self chunks transpose as soon as g0 lands (tail shaving:
            # only the nsum chunks depend on the last gather)
            for k in range(4):
                pt = psum_t.tile([P, P], mybir.dt.bfloat16)
                nc.tensor.transpose(
                    out=pt[:], in_=gs[0][:, k * P : (k + 1) * P],
                    identity=identity[:])
                nc.scalar.copy(out=cts[k][:], in_=pt[:])

            # neighbor sum: chained bf16 adds on DVE
            nsum = spool.tile([P, F], mybir.dt.bfloat16)
            if t < TILES - 1:
                nc.vector.tensor_add(out=nsum[:], in0=gs[1][:], in1=gs[2][:])
                for j in range(3, J):
                    nc.vector.tensor_add(out=nsum[:], in0=nsum[:], in1=gs[j][:])
                if t + 3 < TILES:
                    stage_tile(t + 3)
                for k in range(4, KCHUNKS):
                    pt = psum_t.tile([P, P], mybir.dt.bfloat16)
                    nc.tensor.transpose(
                        out=pt[:], in_=nsum[:, (k - 4) * P : (k - 3) * P],
                        identity=identity[:])
                    nc.scalar.copy(out=cts[k][:], in_=pt[:])
            else:
                # last tile: per-128-col chains so each nsum chunk's
                # transpose fires right after its own final add
                for c in range(4):
                    sl = slice(c * P, (c + 1) * P)
                    nc.vector.tensor_add(
                        out=nsum[:, sl], in0=gs[1][:, sl], in1=gs[2][:, sl])
                    for j in range(3, J):
                        nc.vector.tensor_add(
                            out=nsum[:, sl], in0=nsum[:, sl], in1=gs[j][:, sl])
                    pt = psum_t.tile([P, P], mybir.dt.bfloat16)
                    nc.tensor.transpose(
                        out=pt[:], in_=nsum[:, sl], identity=identity[:])
                    nc.scalar.copy(out=cts[4 + c][:], in_=pt[:])

            po = psum_o.tile([E, P], mybir.dt.float32)
            for k in range(KCHUNKS):
                nc.tensor.matmul(
                    out=po[:],
                    lhsT=wt_sb[:, k * E : (k + 1) * E],
                    rhs=cts[k][:],
                    start=(k == 0),
                    stop=(k == KCHUNKS - 1),
                )

            nc.scalar.activation(
                out=out_sb[:, t * P : (t + 1) * P],
                in_=po[:],
                func=mybir.ActivationFunctionType.Relu,
            )
            # per-tile store overlaps with later gathers
            nc.sync.dma_start(
                out=out[:, t * P : (t + 1) * P],
                in_=out_sb[:, t * P : (t + 1) * P],
            )

    nc.compile()
    return nc


def _get_nc():
    if "nc" not in _CACHE:
        _CACHE["nc"] = build_nc()
    return _CACHE["nc"]


def make_in_maps(nodes, neigh_idx, features, weight):
    nodes = np.asarray(nodes, dtype=np.int32)
    neigh_idx = np.asarray(neigh_idx, dtype=np.int32)
    features = np.ascontiguousarray(np.asarray(features, dtype=np.float32)).astype(BF)
    weight = np.asarray(weight, dtype=np.float32)

    gidx = np.zeros((B, IDXW), dtype=np.int32)
    gidx[:, 0] = nodes
    gidx[:, 1 : J] = neigh_idx
    # prelay per core: sbuf layout [p, (t w)] where node n = t*P + p
    gidx = gidx.reshape(NCORES, TILES, P, IDXW).transpose(0, 2, 1, 3).reshape(
        NCORES, P, TILES * IDXW)

    w = weight.copy()
    w[:, F:] *= 1.0 / S
    wt = np.ascontiguousarray(w.T).astype(BF)  # [2F, E]
    ident = np.eye(P, dtype=np.float32).astype(BF)

    return [
        {
            "gidx": np.ascontiguousarray(gidx[c]),
            "features": features,
            "wt": wt,
            "ident": ident,
        }
        for c in range(NCORES)
    ]


def run(nodes, neigh_idx, features, weight, trace=False):
    nc = _get_nc()
    in_maps = make_in_maps(nodes, neigh_idx, features, weight)
    res = run_bass_kernel_spmd(nc, in_maps, list(range(NCORES)), trace=trace)
    full = np.concatenate([res.results[c]["out"] for c in range(NCORES)], axis=1)
    return full, res


def kernel(nodes, neigh_idx, features, weight):
    full, _ = run(nodes, neigh_idx, features, weight, trace=False)
    return full
